# revision 43
# baseline (speedup 1.0000x reference)
"""Trainium2 Bass kernel for KPToSkl: keypoint-skeleton Gaussian heatmap.

Computes heat[b,y,x] = max_e exp(-gamma * dist^2(pixel, segment_e))
                     = exp(-gamma * min_e dist^2)   (exp is monotone)

Data-parallel over batch: B=32 split as 4 batches per NeuronCore x 8 cores.

Math per (b, e), over the 256x256 image laid out as [128 partitions
(y mod 128), 512 free (yhalf*256 + x)]:

  PE:  As = sqrt(g)*(a - s/2)  where a = projection length along the edge,
       s = edge length: affine in (y, x) -> rank-k matmul vs a shared
       x-basis.  Segment band <=> |As| <= sigma (= sqrt(g)*s/2).
       W2 = gamma * perp_dist^2: quadratic in (y, x) -> rank-k matmul.
  ACT: z = |As|                       (Abs, batched over an edge pair)
  DVE: o = max(z, sigma) - sigma      (dual-op tensor_scalar, per-edge col)
  ACT/DVE: q = o^2 written straight into the d2 PSUM bank (fp32)
  PE:  W2 matmul with start=False accumulates onto q: d2 = q + W2
       (has_written bits persist from 4 one-time dummy matmuls)
  DVE: M = min(M, d2);  ACT: heat = exp(-M) at the end.

All matmul weight sets are zero-padded to k=128 contraction rows (3 edge
pairs x 34 coefficient rows per set): on this silicon k<128 keeps the PE
clock-gated at 1.2 GHz, k=128 runs 2.4 GHz (216 ns / 512-col matmul).
Coefficients are fp16 hi/lo pairs against an exact fp16 basis (x/256,
its square split hi/lo), giving ~1e-4 overall accuracy.
"""

from contextlib import ExitStack

import numpy as np

import concourse.bass as bass
import concourse.tile as tile
from concourse import bacc, mybir
from concourse.bass_utils import run_bass_kernel_spmd

N_CORES = 8
B_TOTAL = 32
B_LOCAL = B_TOTAL // N_CORES  # 4
E = 18
H = W = 256
GAMMA = 0.2
SG = float(np.sqrt(GAMMA))
BE = B_LOCAL * E          # 72 edges per core
NPAIR = BE // 2           # 36
PAIRS_PER_SET = 3
NGROUP = NPAIR // PAIRS_PER_SET  # 12 weight sets

KA = 6                    # lhsT rows for the A matmul
KW = 11                   # lhsT rows for the W2 matmul
KE = KA + KW              # 17 rows per edge, 34 per pair
KPAD = 128                # zero-padded contraction depth

DVE_Q_PAIRS = 4           # of the 36 pairs, this many do q = o*o on DVE
O_ON_GPSIMD = False       # GPSIMD tensor_scalar measured at 7.6us/op — unusable
OA_MOD = 10**9                # every OA_MOD-th edge computes o = relu(z-sig) on ACT

F16 = mybir.dt.float16
F32 = mybir.dt.float32

_cache = {}


def _split16(v):
    v = np.asarray(v, np.float64)
    hi = v.astype(np.float16)
    lo = (v - hi.astype(np.float64)).astype(np.float16)
    return hi, lo


def _basis_tables():
    """12 zero-padded [128, 512] rhs variants: (pair slot 0..2) x
    (A half0, W half0, A half1, W half1).

    x basis is x/256 (exact in fp16); x^2 basis is split hi/lo.  Rescaling
    keeps per-edge coefficients in fp16's normal range (subnormals have an
    absolute error floor that x^2 ~ 65e3 would amplify to ~2e-3).
    """
    x = np.arange(W, dtype=np.float64)
    xs = x / 256.0
    ones = np.ones(W, np.float64)
    zero = np.zeros(W, np.float64)
    xs2 = xs * xs
    xs2h, xs2l = _split16(xs2)
    xs2h = xs2h.astype(np.float64)
    xs2l = xs2l.astype(np.float64)

    def row(a, b):
        return np.concatenate([a, b])

    basisA = np.stack([
        row(xs, xs),      # c1_hi
        row(xs, xs),      # c1_lo
        row(ones, zero),  # d_hi yt0
        row(ones, zero),  # d_lo yt0
        row(zero, ones),  # d_hi yt1
        row(zero, ones),  # d_lo yt1
    ])
    basisW = np.stack([
        row(xs2h, xs2h),  # C_hi
        row(xs2l, xs2l),  # C_hi
        row(xs2h, xs2h),  # C_lo
        row(xs, zero),    # f_hi yt0
        row(xs, zero),    # f_lo yt0
        row(zero, xs),    # f_hi yt1
        row(zero, xs),    # f_lo yt1
        row(ones, zero),  # g_hi yt0
        row(ones, zero),  # g_lo yt0
        row(zero, ones),  # g_hi yt1
        row(zero, ones),  # g_lo yt1
    ])
    variants = []
    for slot in range(PAIRS_PER_SET):
        for half in (0, 1):
            for mat, off in ((basisA, 0), (basisW, KA)):
                v = np.zeros((KPAD, 512), np.float64)
                base = slot * 2 * KE + half * KE + off
                v[base:base + mat.shape[0], :] = mat
                variants.append(v.astype(np.float16))
    return np.stack(variants)


def _core_tables(kps_core):
    """Per-core packed lhsT table [KPAD, NGROUP*128] + sigma cols."""
    ky = kps_core[:, :, 1].astype(np.float64)
    kx = kps_core[:, :, 0].astype(np.float64)
    PI = np.arange(E)
    PJ = (np.arange(E) + 1) % E
    piy, pix = ky[:, PI], kx[:, PI]
    pjy, pjx = ky[:, PJ], kx[:, PJ]
    vy, vx = piy - pjy, pix - pjx
    vn = np.maximum(vy * vy + vx * vx, 1e-12)
    s = np.sqrt(vn)

    # As = SG*(a - s/2), a = ((yc-pjy)vy + (xc-pjx)vx)/s
    Pc = SG * vy / s
    Qc = SG * vx / s
    Rc = -SG * ((pjy * vy + pjx * vx) / s + s / 2)
    c1 = Qc * 512.0 / 255.0          # coefficient of xs = x/256
    # w = sqrt(gamma) * perpendicular distance (gamma-folded)
    G = SG * vx / s
    Hc = -SG * vy / s
    J = SG * (pjx * vy - pjy * vx) / s
    c2 = Hc * 512.0 / 255.0
    C = c2 * c2

    p = np.arange(128, dtype=np.float64)
    yc0 = 2 * p / 255.0 - 1
    yc1 = 2 * (p + 128) / 255.0 - 1

    lhsG = np.zeros((KPAD, NGROUP, 128), np.float16)
    sig = np.zeros((128, BE), np.float32)

    for b in range(B_LOCAL):
        for e in range(E):
            be = b * E + e
            pi, half = be // 2, be % 2
            g, slot = pi // PAIRS_PER_SET, pi % PAIRS_PER_SET
            base = slot * 2 * KE + half * KE

            c1h, c1l = _split16(c1[b, e])
            lhsG[base + 0, g, :] = c1h
            lhsG[base + 1, g, :] = c1l
            for yt, yc in ((0, yc0), (1, yc1)):
                d = Pc[b, e] * yc + (Rc[b, e] - Qc[b, e])
                dh, dl = _split16(d)
                lhsG[base + 2 + 2 * yt, g, :] = dh
                lhsG[base + 3 + 2 * yt, g, :] = dl

            wb = base + KA
            Ch, Cl = _split16(C[b, e])
            lhsG[wb + 0, g, :] = Ch
            lhsG[wb + 1, g, :] = Ch
            lhsG[wb + 2, g, :] = Cl
            for yt, yc in ((0, yc0), (1, yc1)):
                ecol = G[b, e] * yc + (J[b, e] - Hc[b, e])
                f = 2 * c2[b, e] * ecol
                gg = ecol * ecol
                fh, fl = _split16(f)
                gh, gl = _split16(gg)
                lhsG[wb + 3 + 2 * yt, g, :] = fh
                lhsG[wb + 4 + 2 * yt, g, :] = fl
                lhsG[wb + 7 + 2 * yt, g, :] = gh
                lhsG[wb + 8 + 2 * yt, g, :] = gl

            sig[:, be] = SG * s[b, e] / 2

    return lhsG.reshape(KPAD, NGROUP * 128), sig


def _build_program():
    nc = bacc.Bacc(
        "TRN2",
        target_bir_lowering=False,
        debug=False,
        num_devices=N_CORES,
    )

    lhsG_d = nc.declare_dram_parameter("lhsG", [KPAD, NGROUP * 128], F16, isOutput=False)
    rhs_d = nc.declare_dram_parameter("rhs", [12, KPAD, 512], F16, isOutput=False)
    sig_d = nc.declare_dram_parameter("sig", [128, BE], F32, isOutput=False)
    nsig_d = nc.declare_dram_parameter("nsig", [128, BE], F32, isOutput=False)
    out_d = nc.declare_dram_parameter("out", [B_LOCAL, H, W], F32, isOutput=True)

    # which pairs run q = o*o on DVE instead of ACT (engine balancing)
    dve_q = set()
    if DVE_Q_PAIRS > 0:
        step = NPAIR / DVE_Q_PAIRS
        dve_q = {int(i * step) for i in range(DVE_Q_PAIRS)}

    with tile.TileContext(nc) as tc, ExitStack() as ctx:
        const = ctx.enter_context(tc.tile_pool(name="const", bufs=1))
        psum = ctx.enter_context(tc.tile_pool(name="psum", bufs=1, space="PSUM"))
        work = ctx.enter_context(tc.tile_pool(name="work", bufs=4))
        mpool = ctx.enter_context(tc.tile_pool(name="mins", bufs=4))
        opool = ctx.enter_context(tc.tile_pool(name="outs", bufs=2))

        # Chunked const loads: each weight group / rhs variant is its own
        # tile + DMA, interleaved so the first pairs' inputs land first and
        # spread over four DGE queues so they run in parallel.
        dmae = [nc.sync, nc.scalar, nc.gpsimd]
        dmac = [0]
        def dma(dst, src):
            dmae[dmac[0] % len(dmae)].dma_start(dst, src)
            dmac[0] += 1

        lhsG_ap = lhsG_d.ap()
        rhs_ap = rhs_d.ap()
        lgs = [const.tile([KPAD, 128], F16, name=f"lg{g}") for g in range(NGROUP)]
        rvs = [const.tile([KPAD, 512], F16, name=f"rv{v}") for v in range(12)]
        sig_t = const.tile([128, BE], F32)
        nsig_t = const.tile([128, BE], F32)
        # first pair's inputs lead the queueing order
        dma(lgs[0][:], lhsG_ap[:, 0:128])
        for v in range(4):
            dma(rvs[v][:], rhs_ap[v])
        dma(sig_t[:], sig_d.ap())
        dma(nsig_t[:], nsig_d.ap())
        for g in range(1, NGROUP):
            dma(lgs[g][:], lhsG_ap[:, g * 128:(g + 1) * 128])
        for v in range(4, 12):
            dma(rvs[v][:], rhs_ap[v])
        zcol = const.tile([128, 1], F32)
        nc.gpsimd.memset(zcol[:], 0.0)
        rz_t = const.tile([KPAD, 512], F16)
        nc.gpsimd.memset(rz_t[:], 0.0)

        def rhs_v(slot, half, kind):  # kind 0=A, 1=W
            return rvs[slot * 4 + half * 2 + kind]

        # Persistent PSUM slots.  The 4 one-time dummy matmuls set the
        # has_written bits of the d2 banks; afterwards q is ACT/DVE-written
        # into the bank (plain write, bits survive) and the W2 matmul with
        # start=False accumulates on top: d2 = q + W2.
        aas = [psum.tile([128, 1024], F32, name=f"aa{i}") for i in range(2)]
        pds = [psum.tile([128, 1024], F32, name=f"pd{i}") for i in range(2)]
        for j in range(4):
            nc.tensor.matmul(
                pds[j // 2][:, (j % 2) * 512:(j % 2 + 1) * 512],
                rz_t[:, 0:128], rz_t[:],
                start=True, stop=True, skip_group_check=True,
            )

        out_ap = out_d.ap()
        pcount = 0
        for b in range(B_LOCAL):
            # Two interleaved min-chains (even/odd edge of each pair) held in
            # one [128,1024] tile so each pair needs a single TT min; merged
            # at the end.  Ping-pong to avoid in-place DVE read/write.
            m0 = mpool.tile([128, 1024], F32, tag="m0")
            m1 = mpool.tile([128, 1024], F32, tag="m1")
            nc.gpsimd.memset(m0[:], 30.0)
            cur, nxt = m0, m1
            for ep in range(E // 2):
                pi = b * (E // 2) + ep
                g, slot = pi // PAIRS_PER_SET, pi % PAIRS_PER_SET
                lhs = lgs[g][:]
                aa = aas[pcount % 2]
                pd = pds[pcount % 2]
                pcount += 1
                for half in (0, 1):
                    nc.tensor.matmul(
                        aa[:, half * 512:(half + 1) * 512], lhs,
                        rhs_v(slot, half, 0)[:],
                        start=True, stop=True, skip_group_check=True,
                    )
                zt = work.tile([128, 1024], F32, tag="zt")
                nc.scalar.activation(
                    zt[:], aa[:], mybir.ActivationFunctionType.Abs,
                    bias=zcol[:], scale=1.0,
                )
                ot = work.tile([128, 1024], F32, tag="ot")
                for half in (0, 1):
                    be = 2 * pi + half
                    oh = ot[:, half * 512:(half + 1) * 512]
                    zh = zt[:, half * 512:(half + 1) * 512]
                    if (be % OA_MOD) == OA_MOD - 1:
                        nc.scalar.activation(
                            oh, zh, mybir.ActivationFunctionType.Relu,
                            bias=nsig_t[:, be:be + 1], scale=1.0,
                        )
                    else:
                        # o = relu(z - sigma); immediate second scalar
                        nc.vector.tensor_scalar(
                            oh, zh, sig_t[:, be:be + 1], 0.0,
                            mybir.AluOpType.subtract, mybir.AluOpType.max,
                        )
                if pi in dve_q:
                    nc.vector.scalar_tensor_tensor(
                        pd[:], ot[:], 1.0, ot[:],
                        mybir.AluOpType.mult, mybir.AluOpType.mult,
                    )
                else:
                    nc.scalar.activation(
                        pd[:], ot[:], mybir.ActivationFunctionType.Square,
                        bias=zcol[:], scale=1.0,
                    )
                for half in (0, 1):
                    nc.tensor.matmul(
                        pd[:, half * 512:(half + 1) * 512], lhs,
                        rhs_v(slot, half, 1)[:],
                        start=False, stop=True, skip_group_check=True,
                    )
                nc.vector.tensor_tensor(
                    nxt[:], cur[:], pd[:], mybir.AluOpType.min
                )
                cur, nxt = nxt, cur
            mf = mpool.tile([128, 512], F32, tag="mf")
            nc.vector.tensor_tensor(
                mf[:], cur[:, 0:512], cur[:, 512:1024], mybir.AluOpType.min
            )
            otile = opool.tile([128, 512], F32)
            nc.scalar.activation(
                otile[:], mf[:], mybir.ActivationFunctionType.Exp,
                bias=zcol[:], scale=-1.0,
            )
            nc.sync.dma_start(out_ap[b, 0:128, :], otile[:, 0:256])
            nc.sync.dma_start(out_ap[b, 128:256, :], otile[:, 256:512])

    nc.compile()
    return nc


def _get_program():
    if "nc" not in _cache:
        _cache["nc"] = _build_program()
    return _cache["nc"]


def _in_maps(kps):
    rhs = _basis_tables()
    in_maps = []
    for c in range(N_CORES):
        lhsG, sig = _core_tables(kps[c * B_LOCAL:(c + 1) * B_LOCAL])
        in_maps.append({"lhsG": lhsG, "rhs": rhs, "sig": sig, "nsig": -sig})
    return in_maps


def kernel(kps: np.ndarray) -> np.ndarray:
    kps = np.asarray(kps, np.float32)
    assert kps.shape == (B_TOTAL, E, 2), kps.shape

    nc = _get_program()
    in_maps = _in_maps(kps)

    last_err = None
    for _attempt in range(3):
        try:
            res = run_bass_kernel_spmd(nc, in_maps, list(range(N_CORES)))
            break
        except Exception as err:  # transient NRT/device hiccups
            last_err = err
    else:
        raise last_err
    out = np.concatenate([res.results[c]["out"] for c in range(N_CORES)], axis=0)
    return out.astype(np.float32)


# revision 44
# speedup vs baseline: 1.0044x; 1.0044x over previous
"""Trainium2 Bass kernel for KPToSkl: keypoint-skeleton Gaussian heatmap.

Computes heat[b,y,x] = max_e exp(-gamma * dist^2(pixel, segment_e))
                     = exp(-gamma * min_e dist^2)   (exp is monotone)

Data-parallel over batch: B=32 split as 4 batches per NeuronCore x 8 cores.

Math per (b, e), over the 256x256 image laid out as [128 partitions
(y mod 128), 512 free (yhalf*256 + x)]:

  PE:  As = sqrt(g)*(a - s/2)  where a = projection length along the edge,
       s = edge length: affine in (y, x) -> rank-k matmul vs a shared
       x-basis.  Segment band <=> |As| <= sigma (= sqrt(g)*s/2).
       W2 = gamma * perp_dist^2: quadratic in (y, x) -> rank-k matmul.
  ACT: z = |As|                       (Abs, batched over an edge pair)
  DVE: o = max(z, sigma) - sigma      (dual-op tensor_scalar, per-edge col)
  ACT/DVE: q = o^2 written straight into the d2 PSUM bank (fp32)
  PE:  W2 matmul with start=False accumulates onto q: d2 = q + W2
       (has_written bits persist from 4 one-time dummy matmuls)
  DVE: M = min(M, d2);  ACT: heat = exp(-M) at the end.

All matmul weight sets are zero-padded to k=128 contraction rows (3 edge
pairs x 34 coefficient rows per set): on this silicon k<128 keeps the PE
clock-gated at 1.2 GHz, k=128 runs 2.4 GHz (216 ns / 512-col matmul).
Coefficients are fp16 hi/lo pairs against an exact fp16 basis (x/256,
its square split hi/lo), giving ~1e-4 overall accuracy.
"""

from contextlib import ExitStack

import numpy as np

import concourse.bass as bass
import concourse.tile as tile
from concourse import bacc, mybir
from concourse.bass_utils import run_bass_kernel_spmd

N_CORES = 8
B_TOTAL = 32
B_LOCAL = B_TOTAL // N_CORES  # 4
E = 18
H = W = 256
GAMMA = 0.2
SG = float(np.sqrt(GAMMA))
BE = B_LOCAL * E          # 72 edges per core
NPAIR = BE // 2           # 36
PAIRS_PER_SET = 3
NGROUP = NPAIR // PAIRS_PER_SET  # 12 weight sets

KA = 6                    # lhsT rows for the A matmul
KW = 11                   # lhsT rows for the W2 matmul
KE = KA + KW              # 17 rows per edge, 34 per pair
KPAD = 128                # zero-padded contraction depth

DVE_Q_PAIRS = 4           # of the 36 pairs, this many do q = o*o on DVE
O_ON_GPSIMD = False       # GPSIMD tensor_scalar measured at 7.6us/op — unusable
OA_MOD = 10**9                # every OA_MOD-th edge computes o = relu(z-sig) on ACT

F16 = mybir.dt.float16
F32 = mybir.dt.float32

_cache = {}


def _split16(v):
    v = np.asarray(v, np.float64)
    hi = v.astype(np.float16)
    lo = (v - hi.astype(np.float64)).astype(np.float16)
    return hi, lo


def _basis_tables():
    """12 zero-padded [128, 512] rhs variants: (pair slot 0..2) x
    (A half0, W half0, A half1, W half1).

    x basis is x/256 (exact in fp16); x^2 basis is split hi/lo.  Rescaling
    keeps per-edge coefficients in fp16's normal range (subnormals have an
    absolute error floor that x^2 ~ 65e3 would amplify to ~2e-3).
    """
    x = np.arange(W, dtype=np.float64)
    xs = x / 256.0
    ones = np.ones(W, np.float64)
    zero = np.zeros(W, np.float64)
    xs2 = xs * xs
    xs2h, xs2l = _split16(xs2)
    xs2h = xs2h.astype(np.float64)
    xs2l = xs2l.astype(np.float64)

    def row(a, b):
        return np.concatenate([a, b])

    basisA = np.stack([
        row(xs, xs),      # c1_hi
        row(xs, xs),      # c1_lo
        row(ones, zero),  # d_hi yt0
        row(ones, zero),  # d_lo yt0
        row(zero, ones),  # d_hi yt1
        row(zero, ones),  # d_lo yt1
    ])
    basisW = np.stack([
        row(xs2h, xs2h),  # C_hi
        row(xs2l, xs2l),  # C_hi
        row(xs2h, xs2h),  # C_lo
        row(xs, zero),    # f_hi yt0
        row(xs, zero),    # f_lo yt0
        row(zero, xs),    # f_hi yt1
        row(zero, xs),    # f_lo yt1
        row(ones, zero),  # g_hi yt0
        row(ones, zero),  # g_lo yt0
        row(zero, ones),  # g_hi yt1
        row(zero, ones),  # g_lo yt1
    ])
    variants = []
    for slot in range(PAIRS_PER_SET):
        for half in (0, 1):
            for mat, off in ((basisA, 0), (basisW, KA)):
                v = np.zeros((KPAD, 512), np.float64)
                base = slot * 2 * KE + half * KE + off
                v[base:base + mat.shape[0], :] = mat
                variants.append(v.astype(np.float16))
    return np.stack(variants)


def _core_tables(kps_core):
    """Per-core packed lhsT table [KPAD, NGROUP*128] + sigma cols."""
    ky = kps_core[:, :, 1].astype(np.float64)
    kx = kps_core[:, :, 0].astype(np.float64)
    PI = np.arange(E)
    PJ = (np.arange(E) + 1) % E
    piy, pix = ky[:, PI], kx[:, PI]
    pjy, pjx = ky[:, PJ], kx[:, PJ]
    vy, vx = piy - pjy, pix - pjx
    vn = np.maximum(vy * vy + vx * vx, 1e-12)
    s = np.sqrt(vn)

    # As = SG*(a - s/2), a = ((yc-pjy)vy + (xc-pjx)vx)/s
    Pc = SG * vy / s
    Qc = SG * vx / s
    Rc = -SG * ((pjy * vy + pjx * vx) / s + s / 2)
    c1 = Qc * 512.0 / 255.0          # coefficient of xs = x/256
    # w = sqrt(gamma) * perpendicular distance (gamma-folded)
    G = SG * vx / s
    Hc = -SG * vy / s
    J = SG * (pjx * vy - pjy * vx) / s
    c2 = Hc * 512.0 / 255.0
    C = c2 * c2

    p = np.arange(128, dtype=np.float64)
    yc0 = 2 * p / 255.0 - 1
    yc1 = 2 * (p + 128) / 255.0 - 1

    lhsG = np.zeros((KPAD, NGROUP, 128), np.float16)
    sig = np.zeros((128, BE), np.float32)

    for b in range(B_LOCAL):
        for e in range(E):
            be = b * E + e
            pi, half = be // 2, be % 2
            g, slot = pi // PAIRS_PER_SET, pi % PAIRS_PER_SET
            base = slot * 2 * KE + half * KE

            c1h, c1l = _split16(c1[b, e])
            lhsG[base + 0, g, :] = c1h
            lhsG[base + 1, g, :] = c1l
            for yt, yc in ((0, yc0), (1, yc1)):
                d = Pc[b, e] * yc + (Rc[b, e] - Qc[b, e])
                dh, dl = _split16(d)
                lhsG[base + 2 + 2 * yt, g, :] = dh
                lhsG[base + 3 + 2 * yt, g, :] = dl

            wb = base + KA
            Ch, Cl = _split16(C[b, e])
            lhsG[wb + 0, g, :] = Ch
            lhsG[wb + 1, g, :] = Ch
            lhsG[wb + 2, g, :] = Cl
            for yt, yc in ((0, yc0), (1, yc1)):
                ecol = G[b, e] * yc + (J[b, e] - Hc[b, e])
                f = 2 * c2[b, e] * ecol
                gg = ecol * ecol
                fh, fl = _split16(f)
                gh, gl = _split16(gg)
                lhsG[wb + 3 + 2 * yt, g, :] = fh
                lhsG[wb + 4 + 2 * yt, g, :] = fl
                lhsG[wb + 7 + 2 * yt, g, :] = gh
                lhsG[wb + 8 + 2 * yt, g, :] = gl

            sig[:, be] = SG * s[b, e] / 2

    return lhsG.reshape(KPAD, NGROUP * 128), sig


def _build_program():
    nc = bacc.Bacc(
        "TRN2",
        target_bir_lowering=False,
        debug=False,
        num_devices=N_CORES,
    )

    lhsG_d = nc.declare_dram_parameter("lhsG", [KPAD, NGROUP * 128], F16, isOutput=False)
    rhs_d = nc.declare_dram_parameter("rhs", [12, KPAD, 512], F16, isOutput=False)
    sig_d = nc.declare_dram_parameter("sig", [128, BE], F32, isOutput=False)
    nsig_d = nc.declare_dram_parameter("nsig", [128, BE], F32, isOutput=False)
    out_d = nc.declare_dram_parameter("out", [B_LOCAL, H, W], F32, isOutput=True)

    # which pairs run q = o*o on DVE instead of ACT (engine balancing)
    dve_q = set()
    if DVE_Q_PAIRS > 0:
        step = NPAIR / DVE_Q_PAIRS
        dve_q = {int(i * step) for i in range(DVE_Q_PAIRS)}

    with tile.TileContext(nc) as tc, ExitStack() as ctx:
        const = ctx.enter_context(tc.tile_pool(name="const", bufs=1))
        psum = ctx.enter_context(tc.tile_pool(name="psum", bufs=1, space="PSUM"))
        work = ctx.enter_context(tc.tile_pool(name="work", bufs=4))
        mpool = ctx.enter_context(tc.tile_pool(name="mins", bufs=4))
        opool = ctx.enter_context(tc.tile_pool(name="outs", bufs=2))

        # Chunked const loads: each weight group / rhs variant is its own
        # tile + DMA, interleaved so the first pairs' inputs land first and
        # spread over four DGE queues so they run in parallel.
        dmae = [nc.sync, nc.scalar, nc.gpsimd]
        dmac = [0]
        def dma(dst, src):
            dmae[dmac[0] % len(dmae)].dma_start(dst, src)
            dmac[0] += 1

        lhsG_ap = lhsG_d.ap()
        rhs_ap = rhs_d.ap()
        lgs = [const.tile([KPAD, 128], F16, name=f"lg{g}") for g in range(NGROUP)]
        rvs = [const.tile([KPAD, 512], F16, name=f"rv{v}") for v in range(12)]
        sig_t = const.tile([128, BE], F32)
        nsig_t = const.tile([128, BE], F32)
        # first pair's inputs lead the queueing order
        dma(lgs[0][:], lhsG_ap[:, 0:128])
        for v in range(4):
            dma(rvs[v][:], rhs_ap[v])
        dma(sig_t[:], sig_d.ap())
        dma(nsig_t[:], nsig_d.ap())
        for g in range(1, NGROUP):
            dma(lgs[g][:], lhsG_ap[:, g * 128:(g + 1) * 128])
        for v in range(4, 12):
            dma(rvs[v][:], rhs_ap[v])
        zcol = const.tile([128, 1], F32)
        nc.gpsimd.memset(zcol[:], 0.0)
        rz_t = const.tile([KPAD, 512], F16)
        nc.gpsimd.memset(rz_t[:], 0.0)

        def rhs_v(slot, half, kind):  # kind 0=A, 1=W
            return rvs[slot * 4 + half * 2 + kind]

        # Persistent PSUM slots.  The 4 one-time dummy matmuls set the
        # has_written bits of the d2 banks; afterwards q is ACT/DVE-written
        # into the bank (plain write, bits survive) and the W2 matmul with
        # start=False accumulates on top: d2 = q + W2.
        aas = [psum.tile([128, 1024], F32, name=f"aa{i}") for i in range(2)]
        pds = [psum.tile([128, 1024], F32, name=f"pd{i}") for i in range(2)]
        for j in range(4):
            nc.tensor.matmul(
                pds[j // 2][:, (j % 2) * 512:(j % 2 + 1) * 512],
                rz_t[:, 0:128], rz_t[:],
                start=True, stop=True, skip_group_check=True,
            )

        out_ap = out_d.ap()
        pcount = 0
        # Per-batch ping-pong min accumulators; batches interleave in the
        # emission order so consecutive DVE mins belong to independent
        # chains (the DVE queue is strict FIFO — spacing dependent ops
        # apart hides d2 latency).
        chains = []
        for b in range(B_LOCAL):
            m0 = mpool.tile([128, 1024], F32, name=f"m0_{b}")
            m1 = mpool.tile([128, 1024], F32, name=f"m1_{b}")
            nc.gpsimd.memset(m0[:], 30.0)
            chains.append([m0, m1])
        for g3 in range(3):
            for b in range(B_LOCAL):
                g = b * 3 + g3
                lhs = lgs[g][:]
                for slot in range(PAIRS_PER_SET):
                    pi = g * PAIRS_PER_SET + slot
                    aa = aas[pcount % 2]
                    pd = pds[pcount % 2]
                    pcount += 1
                    for half in (0, 1):
                        nc.tensor.matmul(
                            aa[:, half * 512:(half + 1) * 512], lhs,
                            rhs_v(slot, half, 0)[:],
                            start=True, stop=True, skip_group_check=True,
                        )
                    zt = work.tile([128, 1024], F32, tag="zt")
                    nc.scalar.activation(
                        zt[:], aa[:], mybir.ActivationFunctionType.Abs,
                        bias=zcol[:], scale=1.0,
                    )
                    ot = work.tile([128, 1024], F32, tag="ot")
                    for half in (0, 1):
                        be = 2 * pi + half
                        nc.vector.tensor_scalar(
                            ot[:, half * 512:(half + 1) * 512],
                            zt[:, half * 512:(half + 1) * 512],
                            sig_t[:, be:be + 1], 0.0,
                            mybir.AluOpType.subtract, mybir.AluOpType.max,
                        )
                    if pi in dve_q:
                        nc.vector.scalar_tensor_tensor(
                            pd[:], ot[:], 1.0, ot[:],
                            mybir.AluOpType.mult, mybir.AluOpType.mult,
                        )
                    else:
                        nc.scalar.activation(
                            pd[:], ot[:], mybir.ActivationFunctionType.Square,
                            bias=zcol[:], scale=1.0,
                        )
                    for half in (0, 1):
                        nc.tensor.matmul(
                            pd[:, half * 512:(half + 1) * 512], lhs,
                            rhs_v(slot, half, 1)[:],
                            start=False, stop=True, skip_group_check=True,
                        )
                    cur, nxt = chains[b]
                    nc.vector.tensor_tensor(
                        nxt[:], cur[:], pd[:], mybir.AluOpType.min
                    )
                    chains[b] = [nxt, cur]
                if g3 == 2:
                    cur = chains[b][0]
                    mf = mpool.tile([128, 512], F32, tag="mf")
                    nc.vector.tensor_tensor(
                        mf[:], cur[:, 0:512], cur[:, 512:1024],
                        mybir.AluOpType.min,
                    )
                    otile = opool.tile([128, 512], F32)
                    nc.scalar.activation(
                        otile[:], mf[:], mybir.ActivationFunctionType.Exp,
                        bias=zcol[:], scale=-1.0,
                    )
                    nc.sync.dma_start(out_ap[b, 0:128, :], otile[:, 0:256])
                    nc.sync.dma_start(out_ap[b, 128:256, :], otile[:, 256:512])

    nc.compile()
    return nc


def _get_program():
    if "nc" not in _cache:
        _cache["nc"] = _build_program()
    return _cache["nc"]


def _in_maps(kps):
    rhs = _basis_tables()
    in_maps = []
    for c in range(N_CORES):
        lhsG, sig = _core_tables(kps[c * B_LOCAL:(c + 1) * B_LOCAL])
        in_maps.append({"lhsG": lhsG, "rhs": rhs, "sig": sig, "nsig": -sig})
    return in_maps


def kernel(kps: np.ndarray) -> np.ndarray:
    kps = np.asarray(kps, np.float32)
    assert kps.shape == (B_TOTAL, E, 2), kps.shape

    nc = _get_program()
    in_maps = _in_maps(kps)

    last_err = None
    for _attempt in range(3):
        try:
            res = run_bass_kernel_spmd(nc, in_maps, list(range(N_CORES)))
            break
        except Exception as err:  # transient NRT/device hiccups
            last_err = err
    else:
        raise last_err
    out = np.concatenate([res.results[c]["out"] for c in range(N_CORES)], axis=0)
    return out.astype(np.float32)


# revision 45
# speedup vs baseline: 1.0058x; 1.0013x over previous
"""Trainium2 Bass kernel for KPToSkl: keypoint-skeleton Gaussian heatmap.

Computes heat[b,y,x] = max_e exp(-gamma * dist^2(pixel, segment_e))
                     = exp(-gamma * min_e dist^2)   (exp is monotone)

Data-parallel over batch: B=32 split as 4 batches per NeuronCore x 8 cores.

Math per (b, e), over the 256x256 image laid out as [128 partitions
(y mod 128), 512 free (yhalf*256 + x)]:

  PE:  As = sqrt(g)*(a - s/2)  where a = projection length along the edge,
       s = edge length: affine in (y, x) -> rank-k matmul vs a shared
       x-basis.  Segment band <=> |As| <= sigma (= sqrt(g)*s/2).
       W2 = gamma * perp_dist^2: quadratic in (y, x) -> rank-k matmul.
  ACT: z = |As|                       (Abs, batched over an edge pair)
  DVE: o = max(z, sigma) - sigma      (dual-op tensor_scalar, per-edge col)
  ACT/DVE: q = o^2 written straight into the d2 PSUM bank (fp32)
  PE:  W2 matmul with start=False accumulates onto q: d2 = q + W2
       (has_written bits persist from 4 one-time dummy matmuls)
  DVE: M = min(M, d2);  ACT: heat = exp(-M) at the end.

All matmul weight sets are zero-padded to k=128 contraction rows (3 edge
pairs x 34 coefficient rows per set): on this silicon k<128 keeps the PE
clock-gated at 1.2 GHz, k=128 runs 2.4 GHz (216 ns / 512-col matmul).
Coefficients are fp16 hi/lo pairs against an exact fp16 basis (x/256,
its square split hi/lo), giving ~1e-4 overall accuracy.
"""

from contextlib import ExitStack

import numpy as np

import concourse.bass as bass
import concourse.tile as tile
from concourse import bacc, mybir
from concourse.bass_utils import run_bass_kernel_spmd

N_CORES = 8
B_TOTAL = 32
B_LOCAL = B_TOTAL // N_CORES  # 4
E = 18
H = W = 256
GAMMA = 0.2
SG = float(np.sqrt(GAMMA))
BE = B_LOCAL * E          # 72 edges per core
NPAIR = BE // 2           # 36
PAIRS_PER_SET = 3
NGROUP = NPAIR // PAIRS_PER_SET  # 12 weight sets

KA = 6                    # lhsT rows for the A matmul
KW = 11                   # lhsT rows for the W2 matmul
KE = KA + KW              # 17 rows per edge, 34 per pair
KPAD = 128                # zero-padded contraction depth

DVE_Q_PAIRS = 5           # of the 36 pairs, this many do q = o*o on DVE
O_ON_GPSIMD = False       # GPSIMD tensor_scalar measured at 7.6us/op — unusable
OA_MOD = 10**9                # every OA_MOD-th edge computes o = relu(z-sig) on ACT

F16 = mybir.dt.float16
F32 = mybir.dt.float32

_cache = {}


def _split16(v):
    v = np.asarray(v, np.float64)
    hi = v.astype(np.float16)
    lo = (v - hi.astype(np.float64)).astype(np.float16)
    return hi, lo


def _basis_tables():
    """12 zero-padded [128, 512] rhs variants: (pair slot 0..2) x
    (A half0, W half0, A half1, W half1).

    x basis is x/256 (exact in fp16); x^2 basis is split hi/lo.  Rescaling
    keeps per-edge coefficients in fp16's normal range (subnormals have an
    absolute error floor that x^2 ~ 65e3 would amplify to ~2e-3).
    """
    x = np.arange(W, dtype=np.float64)
    xs = x / 256.0
    ones = np.ones(W, np.float64)
    zero = np.zeros(W, np.float64)
    xs2 = xs * xs
    xs2h, xs2l = _split16(xs2)
    xs2h = xs2h.astype(np.float64)
    xs2l = xs2l.astype(np.float64)

    def row(a, b):
        return np.concatenate([a, b])

    basisA = np.stack([
        row(xs, xs),      # c1_hi
        row(xs, xs),      # c1_lo
        row(ones, zero),  # d_hi yt0
        row(ones, zero),  # d_lo yt0
        row(zero, ones),  # d_hi yt1
        row(zero, ones),  # d_lo yt1
    ])
    basisW = np.stack([
        row(xs2h, xs2h),  # C_hi
        row(xs2l, xs2l),  # C_hi
        row(xs2h, xs2h),  # C_lo
        row(xs, zero),    # f_hi yt0
        row(xs, zero),    # f_lo yt0
        row(zero, xs),    # f_hi yt1
        row(zero, xs),    # f_lo yt1
        row(ones, zero),  # g_hi yt0
        row(ones, zero),  # g_lo yt0
        row(zero, ones),  # g_hi yt1
        row(zero, ones),  # g_lo yt1
    ])
    variants = []
    for slot in range(PAIRS_PER_SET):
        for half in (0, 1):
            for mat, off in ((basisA, 0), (basisW, KA)):
                v = np.zeros((KPAD, 512), np.float64)
                base = slot * 2 * KE + half * KE + off
                v[base:base + mat.shape[0], :] = mat
                variants.append(v.astype(np.float16))
    return np.stack(variants)


def _core_tables(kps_core):
    """Per-core packed lhsT table [KPAD, NGROUP*128] + sigma cols."""
    ky = kps_core[:, :, 1].astype(np.float64)
    kx = kps_core[:, :, 0].astype(np.float64)
    PI = np.arange(E)
    PJ = (np.arange(E) + 1) % E
    piy, pix = ky[:, PI], kx[:, PI]
    pjy, pjx = ky[:, PJ], kx[:, PJ]
    vy, vx = piy - pjy, pix - pjx
    vn = np.maximum(vy * vy + vx * vx, 1e-12)
    s = np.sqrt(vn)

    # As = SG*(a - s/2), a = ((yc-pjy)vy + (xc-pjx)vx)/s
    Pc = SG * vy / s
    Qc = SG * vx / s
    Rc = -SG * ((pjy * vy + pjx * vx) / s + s / 2)
    c1 = Qc * 512.0 / 255.0          # coefficient of xs = x/256
    # w = sqrt(gamma) * perpendicular distance (gamma-folded)
    G = SG * vx / s
    Hc = -SG * vy / s
    J = SG * (pjx * vy - pjy * vx) / s
    c2 = Hc * 512.0 / 255.0
    C = c2 * c2

    p = np.arange(128, dtype=np.float64)
    yc0 = 2 * p / 255.0 - 1
    yc1 = 2 * (p + 128) / 255.0 - 1

    lhsG = np.zeros((KPAD, NGROUP, 128), np.float16)
    sig = np.zeros((128, BE), np.float32)

    for b in range(B_LOCAL):
        for e in range(E):
            be = b * E + e
            pi, half = be // 2, be % 2
            g, slot = pi // PAIRS_PER_SET, pi % PAIRS_PER_SET
            base = slot * 2 * KE + half * KE

            c1h, c1l = _split16(c1[b, e])
            lhsG[base + 0, g, :] = c1h
            lhsG[base + 1, g, :] = c1l
            for yt, yc in ((0, yc0), (1, yc1)):
                d = Pc[b, e] * yc + (Rc[b, e] - Qc[b, e])
                dh, dl = _split16(d)
                lhsG[base + 2 + 2 * yt, g, :] = dh
                lhsG[base + 3 + 2 * yt, g, :] = dl

            wb = base + KA
            Ch, Cl = _split16(C[b, e])
            lhsG[wb + 0, g, :] = Ch
            lhsG[wb + 1, g, :] = Ch
            lhsG[wb + 2, g, :] = Cl
            for yt, yc in ((0, yc0), (1, yc1)):
                ecol = G[b, e] * yc + (J[b, e] - Hc[b, e])
                f = 2 * c2[b, e] * ecol
                gg = ecol * ecol
                fh, fl = _split16(f)
                gh, gl = _split16(gg)
                lhsG[wb + 3 + 2 * yt, g, :] = fh
                lhsG[wb + 4 + 2 * yt, g, :] = fl
                lhsG[wb + 7 + 2 * yt, g, :] = gh
                lhsG[wb + 8 + 2 * yt, g, :] = gl

            sig[:, be] = SG * s[b, e] / 2

    return lhsG.reshape(KPAD, NGROUP * 128), sig


def _build_program():
    nc = bacc.Bacc(
        "TRN2",
        target_bir_lowering=False,
        debug=False,
        num_devices=N_CORES,
    )

    lhsG_d = nc.declare_dram_parameter("lhsG", [KPAD, NGROUP * 128], F16, isOutput=False)
    rhs_d = nc.declare_dram_parameter("rhs", [12, KPAD, 512], F16, isOutput=False)
    sig_d = nc.declare_dram_parameter("sig", [128, BE], F32, isOutput=False)
    nsig_d = nc.declare_dram_parameter("nsig", [128, BE], F32, isOutput=False)
    out_d = nc.declare_dram_parameter("out", [B_LOCAL, H, W], F32, isOutput=True)

    # which pairs run q = o*o on DVE instead of ACT (engine balancing)
    dve_q = set()
    if DVE_Q_PAIRS > 0:
        step = NPAIR / DVE_Q_PAIRS
        dve_q = {int(i * step) for i in range(DVE_Q_PAIRS)}

    with tile.TileContext(nc) as tc, ExitStack() as ctx:
        const = ctx.enter_context(tc.tile_pool(name="const", bufs=1))
        psum = ctx.enter_context(tc.tile_pool(name="psum", bufs=1, space="PSUM"))
        work = ctx.enter_context(tc.tile_pool(name="work", bufs=6))
        mpool = ctx.enter_context(tc.tile_pool(name="mins", bufs=4))
        opool = ctx.enter_context(tc.tile_pool(name="outs", bufs=2))

        # Chunked const loads: each weight group / rhs variant is its own
        # tile + DMA, interleaved so the first pairs' inputs land first and
        # spread over four DGE queues so they run in parallel.
        dmae = [nc.sync, nc.scalar, nc.gpsimd]
        dmac = [0]
        def dma(dst, src):
            dmae[dmac[0] % len(dmae)].dma_start(dst, src)
            dmac[0] += 1

        lhsG_ap = lhsG_d.ap()
        rhs_ap = rhs_d.ap()
        lgs = [const.tile([KPAD, 128], F16, name=f"lg{g}") for g in range(NGROUP)]
        rvs = [const.tile([KPAD, 512], F16, name=f"rv{v}") for v in range(12)]
        sig_t = const.tile([128, BE], F32)
        nsig_t = const.tile([128, BE], F32)
        # first pair's inputs lead the queueing order
        dma(lgs[0][:], lhsG_ap[:, 0:128])
        for v in range(4):
            dma(rvs[v][:], rhs_ap[v])
        dma(sig_t[:], sig_d.ap())
        dma(nsig_t[:], nsig_d.ap())
        for g in range(1, NGROUP):
            dma(lgs[g][:], lhsG_ap[:, g * 128:(g + 1) * 128])
        for v in range(4, 12):
            dma(rvs[v][:], rhs_ap[v])
        zcol = const.tile([128, 1], F32)
        nc.gpsimd.memset(zcol[:], 0.0)
        rz_t = const.tile([KPAD, 512], F16)
        nc.gpsimd.memset(rz_t[:], 0.0)

        def rhs_v(slot, half, kind):  # kind 0=A, 1=W
            return rvs[slot * 4 + half * 2 + kind]

        # Persistent PSUM slots.  The 4 one-time dummy matmuls set the
        # has_written bits of the d2 banks; afterwards q is ACT/DVE-written
        # into the bank (plain write, bits survive) and the W2 matmul with
        # start=False accumulates on top: d2 = q + W2.
        aas = [psum.tile([128, 1024], F32, name=f"aa{i}") for i in range(2)]
        pds = [psum.tile([128, 1024], F32, name=f"pd{i}") for i in range(2)]
        for j in range(4):
            nc.tensor.matmul(
                pds[j // 2][:, (j % 2) * 512:(j % 2 + 1) * 512],
                rz_t[:, 0:128], rz_t[:],
                start=True, stop=True, skip_group_check=True,
            )

        out_ap = out_d.ap()
        pcount = 0
        # Per-batch ping-pong min accumulators; batches interleave in the
        # emission order so consecutive DVE mins belong to independent
        # chains (the DVE queue is strict FIFO — spacing dependent ops
        # apart hides d2 latency).
        chains = []
        for b in range(B_LOCAL):
            m0 = mpool.tile([128, 1024], F32, name=f"m0_{b}")
            m1 = mpool.tile([128, 1024], F32, name=f"m1_{b}")
            nc.gpsimd.memset(m0[:], 30.0)
            chains.append([m0, m1])
        for g3 in range(3):
            for b in range(B_LOCAL):
                g = b * 3 + g3
                lhs = lgs[g][:]
                for slot in range(PAIRS_PER_SET):
                    pi = g * PAIRS_PER_SET + slot
                    aa = aas[pcount % 2]
                    pd = pds[pcount % 2]
                    pcount += 1
                    for half in (0, 1):
                        nc.tensor.matmul(
                            aa[:, half * 512:(half + 1) * 512], lhs,
                            rhs_v(slot, half, 0)[:],
                            start=True, stop=True, skip_group_check=True,
                        )
                    zt = work.tile([128, 1024], F32, tag="zt")
                    nc.scalar.activation(
                        zt[:], aa[:], mybir.ActivationFunctionType.Abs,
                        bias=zcol[:], scale=1.0,
                    )
                    ot = work.tile([128, 1024], F32, tag="ot")
                    for half in (0, 1):
                        be = 2 * pi + half
                        nc.vector.tensor_scalar(
                            ot[:, half * 512:(half + 1) * 512],
                            zt[:, half * 512:(half + 1) * 512],
                            sig_t[:, be:be + 1], 0.0,
                            mybir.AluOpType.subtract, mybir.AluOpType.max,
                        )
                    if pi in dve_q:
                        nc.vector.scalar_tensor_tensor(
                            pd[:], ot[:], 1.0, ot[:],
                            mybir.AluOpType.mult, mybir.AluOpType.mult,
                        )
                    else:
                        nc.scalar.activation(
                            pd[:], ot[:], mybir.ActivationFunctionType.Square,
                            bias=zcol[:], scale=1.0,
                        )
                    for half in (0, 1):
                        nc.tensor.matmul(
                            pd[:, half * 512:(half + 1) * 512], lhs,
                            rhs_v(slot, half, 1)[:],
                            start=False, stop=True, skip_group_check=True,
                        )
                    cur, nxt = chains[b]
                    nc.vector.tensor_tensor(
                        nxt[:], cur[:], pd[:], mybir.AluOpType.min
                    )
                    chains[b] = [nxt, cur]
                if g3 == 2:
                    cur = chains[b][0]
                    mf = mpool.tile([128, 512], F32, tag="mf")
                    nc.vector.tensor_tensor(
                        mf[:], cur[:, 0:512], cur[:, 512:1024],
                        mybir.AluOpType.min,
                    )
                    otile = opool.tile([128, 512], F32)
                    nc.scalar.activation(
                        otile[:], mf[:], mybir.ActivationFunctionType.Exp,
                        bias=zcol[:], scale=-1.0,
                    )
                    nc.sync.dma_start(out_ap[b, 0:128, :], otile[:, 0:256])
                    nc.sync.dma_start(out_ap[b, 128:256, :], otile[:, 256:512])

    nc.compile()
    return nc


def _get_program():
    if "nc" not in _cache:
        _cache["nc"] = _build_program()
    return _cache["nc"]


def _in_maps(kps):
    rhs = _basis_tables()
    in_maps = []
    for c in range(N_CORES):
        lhsG, sig = _core_tables(kps[c * B_LOCAL:(c + 1) * B_LOCAL])
        in_maps.append({"lhsG": lhsG, "rhs": rhs, "sig": sig, "nsig": -sig})
    return in_maps


def kernel(kps: np.ndarray) -> np.ndarray:
    kps = np.asarray(kps, np.float32)
    assert kps.shape == (B_TOTAL, E, 2), kps.shape

    nc = _get_program()
    in_maps = _in_maps(kps)

    last_err = None
    for _attempt in range(3):
        try:
            res = run_bass_kernel_spmd(nc, in_maps, list(range(N_CORES)))
            break
        except Exception as err:  # transient NRT/device hiccups
            last_err = err
    else:
        raise last_err
    out = np.concatenate([res.results[c]["out"] for c in range(N_CORES)], axis=0)
    return out.astype(np.float32)
